# revision 5
# baseline (speedup 1.0000x reference)
"""TT-adapter linear kernel for TRN2, data-parallel over batch on 8 NeuronCores.

Math: out = x @ W.T + b + ALPHA * TT(x).  TT is linear in x, so the module
collapses to a single matmul with a merged weight folded on host:

    Wc = W + ALPHA * T          (T = TT-matrix reconstruction, 1024x1024)
    out = x @ Wc.T + b

The 34 GFLOP batched matmul runs on device in bf16 (f32 PSUM accumulation),
one batch element per NeuronCore, no collectives.  Raw bacc (manual
semaphores).  PE floor is 256 MMs x 216 ns = 55.3 us.

Measured DMA behavior that shapes the schedule: one HWDGE queue sustains
~300 GB/s early, ~400 GB/s warm, and a DMA's completion semaphore reaches
+16 only ~0.3 us (small DMA) to ~1.4 us (512 KB w/ backlog) after its first
engine finishes -- so every PE gate costs data-time + sem-spread.  Hence:
small DMAs for anything on the critical path, big DMAs for bulk.

Host layouts (per core, P=128 partitions, contraction dim on partitions):
    wt  bf16 [8, 128, 1024]     wt[d, p, o*128+j] = Wc[o*128+j, 128d+p]
    xs  bf16 [4, 128, 8, 512]   xs[sc, p, d, j]   = x[b, 512sc+j, 128d+p]
    bi  f32  [128, 8]           bi[p, oo]         = b[128oo + p]
    out bf16 [8, 128, 2048]     out[oo, p, s]     = result[b, s, 128oo+p]

Schedule per core (group idx = 8*sc + o; bank = o; all MMs N=512):
  SP:  inputs in arrival-critical order: per-d weight halves (128 KB each,
       small sem-spread) interleaved with the sc=0 x slices, then the bulk
       sc=1..3 x (1 MB each) and bias; then out-DMAs idx=0..30 gated on
       evictions; final wait on the 8 slot sems.
  PE:  10 HAM-warm-up matmuls; phase 1 = sc=0 strip (o=0..7) d-outer
       staircase across all 8 PSUM banks -- needs only 384 KB per d step;
       phase 2 = sc=1..3 strips, d-inner per group, gated on that sc's x
       bulk DMA + bank eviction.
  ACT: dummy 8-col activate first (hoists the lazy 1.3 us ACT_TABLE_LOAD
       into the preamble), then 32 evictions (PSUM -> SBUF bf16 + bias),
       last group's out-DMA ships from ACT directly.
"""

import numpy as np
import ml_dtypes
from contextlib import ExitStack

import concourse.bass as bass  # noqa: F401
import concourse.mybir as mybir
from concourse import bacc
from concourse.bass_utils import run_bass_kernel_spmd

ALPHA = 16.0
B, S, D = 8, 2048, 1024
P = 128
DO = D // P          # 8 contraction tiles
OO = D // P          # 8 output tiles
SCH = 512
NS = S // SCH        # 4 s-chunks
NG = OO * NS         # 32 groups
NBANK = 8
NSLOT = 8

_NC = None


def _build_nc():
    nc = bacc.Bacc("TRN2", target_bir_lowering=False, debug=False)
    wt = nc.declare_dram_parameter("wt", [DO, P, D], mybir.dt.bfloat16, isOutput=False)
    xs = nc.declare_dram_parameter("xs", [NS, P, DO, SCH], mybir.dt.bfloat16, isOutput=False)
    bi = nc.declare_dram_parameter("bi", [P, OO], mybir.dt.float32, isOutput=False)
    out = nc.declare_dram_parameter("out", [OO, P, S], mybir.dt.bfloat16, isOutput=True)

    with ExitStack() as ctx:
        block = ctx.enter_context(nc.Block())
        # One sem per gating granule; HWDGE completion increments +16 per DMA
        # but the 16 per-engine incs straggle, so granules stay small where
        # latency matters.
        s_w = [ctx.enter_context(nc.semaphore(f"s_w{d}")) for d in range(DO)]
        s_x0a = ctx.enter_context(nc.semaphore("s_x0a"))    # xs[0][:, 0:2]
        s_x0b = ctx.enter_context(nc.semaphore("s_x0b"))    # xs[0][:, 2:8]
        s_xs = [ctx.enter_context(nc.semaphore(f"s_xs{sc}")) for sc in range(1, NS)]
        s_bias = ctx.enter_context(nc.semaphore("s_bias"))
        s_mm = ctx.enter_context(nc.semaphore("s_mm"))
        s_ev = ctx.enter_context(nc.semaphore("s_ev"))
        s_slot = [ctx.enter_context(nc.semaphore(f"s_slot{k}")) for k in range(NSLOT)]

        w_sb = ctx.enter_context(nc.sbuf_tensor("w_sb", [P, DO, D], mybir.dt.bfloat16))
        x_sb = ctx.enter_context(nc.sbuf_tensor("x_sb", [P, NS, DO, SCH], mybir.dt.bfloat16))
        bias_sb = ctx.enter_context(nc.sbuf_tensor("bias_sb", [P, OO], mybir.dt.float32))
        ot_sb = ctx.enter_context(nc.sbuf_tensor("ot_sb", [P, NSLOT, SCH], mybir.dt.bfloat16))
        ps = [ctx.enter_context(nc.psum_tensor(f"ps{b}", [P, SCH], mybir.dt.float32))
              for b in range(NBANK)]

        def wsl(o, d):
            return w_sb[:, d, o * P:(o + 1) * P]

        def xsl(sc, d):
            return x_sb[:, sc, d, :]

        @block.sync
        def _(sync: bass.BassEngine):
            # bias first (4KB, unblocks eviction 0 early), then the x stream
            # in need-order; weights flow on ACT's HWDGE queue in parallel
            sync.dma_start(out=bias_sb[:, :], in_=bi[:, :]).then_inc(s_bias, 16)
            sync.dma_start(out=x_sb[:, 0, 0:2, :], in_=xs[0][:, 0:2, :]).then_inc(s_x0a, 16)
            sync.dma_start(out=x_sb[:, 0, 2:DO, :], in_=xs[0][:, 2:DO, :]).then_inc(s_x0b, 16)
            for sc in range(1, NS):
                sync.dma_start(out=x_sb[:, sc, :, :], in_=xs[sc]).then_inc(s_xs[sc - 1], 16)
            for g in range(NG - 1):
                o, sc = g % OO, g // OO
                sync.wait_ge(s_ev, g + 1)
                sync.dma_start(
                    out=out[o, :, sc * SCH:(sc + 1) * SCH],
                    in_=ot_sb[:, g % NSLOT, :],
                ).then_inc(s_slot[g % NSLOT], 16)
            for k in range(NSLOT):
                sync.wait_ge(s_slot[k], 16 * (NG // NSLOT))

        @block.tensor
        def _(tensor: bass.BassEngine):
            # HAM warm-up on whatever is in SBUF during the otherwise-idle
            # preamble/input-latency window; results discarded (group o=0
            # restarts bank 0 with start=True).
            for _ in range(14):
                tensor.matmul(
                    ps[0][:, 0:256],
                    w_sb[:, 0, 0:P],
                    x_sb[:, 0, 0, 0:256],
                    start=True,
                    stop=True,
                )
            # phase 1: sc=0 strip, d-outer staircase over banks 0..7 (=o)
            for d in range(DO):
                tensor.wait_ge(s_w[d], 16)
                if d == 0:
                    tensor.wait_ge(s_x0a, 16)
                elif d == 2:
                    tensor.wait_ge(s_x0b, 16)
                for o in range(OO):
                    mmi = tensor.matmul(
                        ps[o][:, :],
                        wsl(o, d),
                        xsl(0, d),
                        start=(d == 0),
                        stop=(d == DO - 1),
                    )
                    if d == DO - 1:
                        # d=7 octet runs in group order 0..7 -> s_mm incs
                        # arrive in the order the evictions expect
                        mmi.then_inc(s_mm, 1)
            # phase 2: sc=1..3 strips, d-inner per group
            for g in range(NBANK, NG):
                o, sc = g % OO, g // OO
                if o == 0:
                    tensor.wait_ge(s_xs[sc - 1], 16)
                tensor.wait_ge(s_ev, g - NBANK + 1)
                for d in range(DO):
                    mmi = tensor.matmul(
                        ps[o][:, :],
                        wsl(o, d),
                        xsl(sc, d),
                        start=(d == 0),
                        stop=(d == DO - 1),
                    )
                    if d == DO - 1:
                        mmi.then_inc(s_mm, 1)

        @block.scalar
        def _(scalar: bass.BassEngine):
            # weight stream on ACT's own HWDGE queue, parallel to SP's x queue
            for d in range(DO):
                scalar.dma_start(out=w_sb[:, d, :], in_=wt[d]).then_inc(s_w[d], 16)
            # dummy 8-col activate: pulls the lazy ACT_TABLE_LOAD into the
            # preamble window (it otherwise delays the first real eviction
            # by ~1.3us).  Reads garbage; slot 0 is fully overwritten by
            # eviction 0 before any out-DMA reads it.
            scalar.add(ot_sb[:, 0, 0:8], bias_sb[:, 0:8], 0.0)
            for g in range(NG):
                o, sc = g % OO, g // OO
                if g == 0:
                    scalar.wait_ge(s_bias, 16)
                scalar.wait_ge(s_mm, g + 1)
                if g >= NSLOT:
                    scalar.wait_ge(s_slot[g % NSLOT], 16 * (g // NSLOT))
                scalar.add(
                    ot_sb[:, g % NSLOT, :], ps[o][:, :], bias_sb[:, o:o + 1]
                ).then_inc(s_ev, 1)
                if g == NG - 1:
                    # last output ships from ACT (also HWDGE, its own queue):
                    # skips the SP semaphore hop on the critical tail
                    scalar.dma_start(
                        out=out[o, :, sc * SCH:(sc + 1) * SCH],
                        in_=ot_sb[:, g % NSLOT, :],
                    ).then_inc(s_slot[g % NSLOT], 16)

    nc.compile()
    return nc


def _get_nc():
    global _NC
    if _NC is None:
        _NC = _build_nc()
    return _NC


def _merged_weight_T(W, b, core0, core1, core2, core3, core4, core5):
    f8 = np.float64
    A = core0[0].astype(f8)
    Bm = np.einsum('ap,pbq->abq', A, core1.astype(f8))
    C = np.einsum('abq,qcr->abcr', Bm, core2.astype(f8))
    Phi = C.transpose(2, 1, 0, 3).reshape(D, 8)
    Dn = np.einsum('paq,qbr->pabr', core3.astype(f8), core4.astype(f8))
    E = np.einsum('pabq,qc->pabc', Dn, core5[:, :, 0].astype(f8))
    Psi = E.reshape(8, D)
    WcT = W.T.astype(f8) + ALPHA * (Phi @ Psi)
    return WcT.astype(np.float32)


def _prep_in_maps(x, W, b, core0, core1, core2, core3, core4, core5):
    WcT = _merged_weight_T(W, b, core0, core1, core2, core3, core4, core5)
    wt = WcT.reshape(DO, P, D).astype(ml_dtypes.bfloat16)
    bi = np.ascontiguousarray(b.reshape(OO, P).T).astype(np.float32)
    in_maps = []
    for bb in range(B):
        xt = x[bb].T.reshape(DO, P, NS, SCH)
        xsc = np.ascontiguousarray(xt.transpose(2, 1, 0, 3)).astype(ml_dtypes.bfloat16)
        in_maps.append({"wt": wt, "xs": xsc, "bi": bi})
    return in_maps


def _gather(results):
    outs = []
    for bb in range(B):
        o = np.asarray(results[bb]["out"]).astype(np.float32)
        outs.append(o.transpose(2, 0, 1).reshape(S, D))
    return np.ascontiguousarray(np.stack(outs))


def run(inputs, **spmd_kwargs):
    inputs = {k: np.asarray(v) for k, v in inputs.items()}
    in_maps = _prep_in_maps(**inputs)
    nc = _get_nc()
    res = run_bass_kernel_spmd(nc, in_maps, core_ids=list(range(B)), **spmd_kwargs)
    return _gather(res.results), res


def kernel(x, W, b, core0, core1, core2, core3, core4, core5):
    out, _ = run(dict(x=x, W=W, b=b, core0=core0, core1=core1, core2=core2,
                      core3=core3, core4=core4, core5=core5))
    return out


# revision 6
# speedup vs baseline: 1.0701x; 1.0701x over previous
"""TT-adapter linear kernel for TRN2, data-parallel over batch on 8 NeuronCores.

Math: out = x @ W.T + b + ALPHA * TT(x).  TT is linear in x, so the module
collapses to a single matmul with a merged weight folded on host:

    Wc = W + ALPHA * T          (T = TT-matrix reconstruction, 1024x1024)
    out = x @ Wc.T + b

The 34 GFLOP batched matmul runs on device in bf16 (f32 PSUM accumulation),
one batch element per NeuronCore, no collectives.  Raw bacc (manual
semaphores).  PE floor is 256 MMs x 216 ns = 55.3 us.

Measured DMA behavior that shapes the schedule: ONE HWDGE queue, in issue
order, sustains ~280 GB/s early / ~400 GB/s warm; splitting across two
queues halves the rate of the critical stream (queues share SDMA engines
without priority), so everything goes on SP's queue in need-order.  A DMA's
completion sem reaches +16 only 0.3 us (128 KB) .. 1.4 us (512 KB, backlog)
after its data lands, so critical-path granules are small and DRAM-side
contiguous (host repacks them).

Host layouts (per core, P=128 partitions, contraction dim on partitions):
    wt   bf16 [8, 128, 1024]    wt[d, p, o*128+j] = Wc[o*128+j, 128d+p]
    w0p  bf16 [2, 128, 512]     wt[0] split into contiguous halves
    xs0p bf16 [8, 128, 512]     xs0p[d, p, j] = x[b, j, 128d+p]   (sc=0)
    xs   bf16 [3, 128, 8, 512]  xs[i, p, d, j] = x[b, 512(i+1)+j, 128d+p]
    bi   f32  [128, 8]          bi[p, oo] = b[128oo + p]
    out  bf16 [8, 128, 2048]    out[oo, p, s] = result[b, s, 128oo+p]

Schedule per core (group idx = 8*sc + o; bank = o; all MMs N=512):
  SP:  single queue in need-order: bias, w0-halves + per-d sc0 x slices
       interleaved with w1..w7, then the sc=1..3 x bulk; out-DMAs idx=0..30
       gated on evictions; final wait on the 8 slot sems.
  PE:  12 HAM-warm-up matmuls cover the preamble->first-data window;
       phase 1 = sc=0 strip (o=0..7) d-outer staircase across all 8 PSUM
       banks (384 KB supply per d step); phase 2 = sc=1..3 strips, d-inner
       per group; each group's gate waits are hoisted before the previous
       group's last MM so the NX resolves them while the PE drains.
  ACT: dummy 8-col activate (hoists the lazy 1.3 us ACT_TABLE_LOAD into
       the preamble), 32 evictions (PSUM -> SBUF bf16 + bias add), last
       group's out-DMA ships from ACT directly (skips the SP sem hop).
"""

import numpy as np
import ml_dtypes
from contextlib import ExitStack

import concourse.bass as bass  # noqa: F401
import concourse.mybir as mybir
from concourse import bacc
from concourse.bass_utils import run_bass_kernel_spmd

ALPHA = 16.0
B, S, D = 8, 2048, 1024
P = 128
DO = D // P          # 8 contraction tiles
OO = D // P          # 8 output tiles
SCH = 512
NS = S // SCH        # 4 s-chunks
NG = OO * NS         # 32 groups
NBANK = 8
NSLOT = 8

_NC = None


def _build_nc():
    nc = bacc.Bacc("TRN2", target_bir_lowering=False, debug=False)
    wt = nc.declare_dram_parameter("wt", [DO, P, D], mybir.dt.bfloat16, isOutput=False)
    w0p = nc.declare_dram_parameter("w0p", [2, P, D // 2], mybir.dt.bfloat16, isOutput=False)
    xs0p = nc.declare_dram_parameter("xs0p", [DO, P, SCH], mybir.dt.bfloat16, isOutput=False)
    xs = nc.declare_dram_parameter("xs", [NS - 1, P, DO, SCH], mybir.dt.bfloat16, isOutput=False)
    bi = nc.declare_dram_parameter("bi", [P, OO], mybir.dt.float32, isOutput=False)
    out = nc.declare_dram_parameter("out", [OO, P, S], mybir.dt.bfloat16, isOutput=True)

    with ExitStack() as ctx:
        block = ctx.enter_context(nc.Block())
        # One sem per gating granule (HWDGE completions are unordered across
        # DMAs, and each DMA incs its sem by 16 -- one per SDMA engine).
        s_w0 = [ctx.enter_context(nc.semaphore(f"s_w0{h}")) for h in range(2)]
        s_w = [ctx.enter_context(nc.semaphore(f"s_w{d}")) for d in range(1, DO)]
        s_x0 = [ctx.enter_context(nc.semaphore(f"s_x0{d}")) for d in range(DO)]
        s_xs = [ctx.enter_context(nc.semaphore(f"s_xs{sc}")) for sc in range(1, NS)]
        s_bias = ctx.enter_context(nc.semaphore("s_bias"))
        s_mm = ctx.enter_context(nc.semaphore("s_mm"))
        s_ev = ctx.enter_context(nc.semaphore("s_ev"))
        s_slot = [ctx.enter_context(nc.semaphore(f"s_slot{k}")) for k in range(NSLOT)]

        w_sb = ctx.enter_context(nc.sbuf_tensor("w_sb", [P, DO, D], mybir.dt.bfloat16))
        x_sb = ctx.enter_context(nc.sbuf_tensor("x_sb", [P, NS, DO, SCH], mybir.dt.bfloat16))
        bias_sb = ctx.enter_context(nc.sbuf_tensor("bias_sb", [P, OO], mybir.dt.float32))
        ot_sb = ctx.enter_context(nc.sbuf_tensor("ot_sb", [P, NSLOT, SCH], mybir.dt.bfloat16))
        ps = [ctx.enter_context(nc.psum_tensor(f"ps{b}", [P, SCH], mybir.dt.float32))
              for b in range(NBANK)]

        def wsl(o, d):
            return w_sb[:, d, o * P:(o + 1) * P]

        def xsl(sc, d):
            return x_sb[:, sc, d, :]

        @block.sync
        def _(sync: bass.BassEngine):
            # need-order: bias (unblocks eviction 0), first-octet granules,
            # then the per-d staircase feed, then the phase-2 bulk
            sync.dma_start(out=bias_sb[:, :], in_=bi[:, :]).then_inc(s_bias, 16)
            sync.dma_start(out=w_sb[:, 0, 0:D // 2], in_=w0p[0]).then_inc(s_w0[0], 16)
            sync.dma_start(out=x_sb[:, 0, 0, :], in_=xs0p[0]).then_inc(s_x0[0], 16)
            sync.dma_start(out=w_sb[:, 0, D // 2:D], in_=w0p[1]).then_inc(s_w0[1], 16)
            for d in range(1, DO):
                sync.dma_start(out=x_sb[:, 0, d, :], in_=xs0p[d]).then_inc(s_x0[d], 16)
                sync.dma_start(out=w_sb[:, d, :], in_=wt[d]).then_inc(s_w[d - 1], 16)
            for sc in range(1, NS):
                sync.dma_start(out=x_sb[:, sc, :, :], in_=xs[sc - 1]).then_inc(s_xs[sc - 1], 16)
            for g in range(NG - 1):
                o, sc = g % OO, g // OO
                sync.wait_ge(s_ev, g + 1)
                sync.dma_start(
                    out=out[o, :, sc * SCH:(sc + 1) * SCH],
                    in_=ot_sb[:, g % NSLOT, :],
                ).then_inc(s_slot[g % NSLOT], 16)
            for k in range(NSLOT):
                sync.wait_ge(s_slot[k], 16 * (NG // NSLOT))

        @block.tensor
        def _(tensor: bass.BassEngine):
            # HAM warm-up on whatever is in SBUF during the otherwise-idle
            # preamble/first-data window; results discarded (bank 0 restarts
            # with start=True).
            for _ in range(12):
                tensor.matmul(
                    ps[0][:, 0:256],
                    w_sb[:, 0, 0:P],
                    x_sb[:, 0, 0, 0:256],
                    start=True,
                    stop=True,
                )
            # phase 1: sc=0 strip, d-outer staircase over banks 0..7 (=o)
            for d in range(DO):
                if d == 0:
                    tensor.wait_ge(s_w0[0], 16)
                    tensor.wait_ge(s_x0[0], 16)
                else:
                    tensor.wait_ge(s_w[d - 1], 16)
                    tensor.wait_ge(s_x0[d], 16)
                for o in range(OO):
                    if d == 0 and o == OO // 2:
                        tensor.wait_ge(s_w0[1], 16)
                    mmi = tensor.matmul(
                        ps[o][:, :],
                        wsl(o, d),
                        xsl(0, d),
                        start=(d == 0),
                        stop=(d == DO - 1),
                    )
                    if d == DO - 1:
                        # d=7 octet runs in group order 0..7 -> s_mm incs
                        # arrive in the order the evictions expect
                        mmi.then_inc(s_mm, 1)
            # phase 2: sc=1..3 strips, d-inner per group.  Group g's gate
            # waits are emitted before the previous group's LAST matmul, so
            # the NX resolves them while the PE is still streaming -- the
            # group-lead MM then issues back-to-back.  Safe: the waited-on
            # eviction (g-8) completed ~12 us earlier.
            for g in range(NBANK, NG):
                o, sc = g % OO, g // OO
                if g == NBANK:
                    tensor.wait_ge(s_xs[sc - 1], 16)
                    tensor.wait_ge(s_ev, g - NBANK + 1)
                for d in range(DO):
                    if d == DO - 1 and g + 1 < NG:
                        no, nsc = (g + 1) % OO, (g + 1) // OO
                        if no == 0:
                            tensor.wait_ge(s_xs[nsc - 1], 16)
                        tensor.wait_ge(s_ev, g + 1 - NBANK + 1)
                    mmi = tensor.matmul(
                        ps[o][:, :],
                        wsl(o, d),
                        xsl(sc, d),
                        start=(d == 0),
                        stop=(d == DO - 1),
                    )
                    if d == DO - 1:
                        mmi.then_inc(s_mm, 1)

        @block.scalar
        def _(scalar: bass.BassEngine):
            # dummy 8-col activate: pulls the lazy ACT_TABLE_LOAD into the
            # preamble window (it otherwise delays the first real eviction
            # by ~1.3us).  Reads garbage; slot 0 is fully overwritten by
            # eviction 0 before any out-DMA reads it.
            scalar.add(ot_sb[:, 0, 0:8], bias_sb[:, 0:8], 0.0)
            for g in range(NG):
                o, sc = g % OO, g // OO
                if g == 0:
                    scalar.wait_ge(s_bias, 16)
                scalar.wait_ge(s_mm, g + 1)
                if g >= NSLOT:
                    scalar.wait_ge(s_slot[g % NSLOT], 16 * (g // NSLOT))
                scalar.add(
                    ot_sb[:, g % NSLOT, :], ps[o][:, :], bias_sb[:, o:o + 1]
                ).then_inc(s_ev, 1)
                if g == NG - 1:
                    # last output ships from ACT (also HWDGE, its own queue):
                    # skips the SP semaphore hop on the critical tail
                    scalar.dma_start(
                        out=out[o, :, sc * SCH:(sc + 1) * SCH],
                        in_=ot_sb[:, g % NSLOT, :],
                    ).then_inc(s_slot[g % NSLOT], 16)

    nc.compile()
    return nc


def _get_nc():
    global _NC
    if _NC is None:
        _NC = _build_nc()
    return _NC


def _merged_weight_T(W, b, core0, core1, core2, core3, core4, core5):
    f8 = np.float64
    A = core0[0].astype(f8)
    Bm = np.einsum('ap,pbq->abq', A, core1.astype(f8))
    C = np.einsum('abq,qcr->abcr', Bm, core2.astype(f8))
    Phi = C.transpose(2, 1, 0, 3).reshape(D, 8)
    Dn = np.einsum('paq,qbr->pabr', core3.astype(f8), core4.astype(f8))
    E = np.einsum('pabq,qc->pabc', Dn, core5[:, :, 0].astype(f8))
    Psi = E.reshape(8, D)
    WcT = W.T.astype(f8) + ALPHA * (Phi @ Psi)
    return WcT.astype(np.float32)


def _prep_in_maps(x, W, b, core0, core1, core2, core3, core4, core5):
    WcT = _merged_weight_T(W, b, core0, core1, core2, core3, core4, core5)
    wt = WcT.reshape(DO, P, D).astype(ml_dtypes.bfloat16)
    w0p = np.ascontiguousarray(wt[0].reshape(P, 2, D // 2).transpose(1, 0, 2))
    bi = np.ascontiguousarray(b.reshape(OO, P).T).astype(np.float32)
    in_maps = []
    for bb in range(B):
        xt = x[bb].T.reshape(DO, P, NS, SCH)          # [d, p, sc, j]
        xs0p = np.ascontiguousarray(xt[:, :, 0, :]).astype(ml_dtypes.bfloat16)
        xsb = np.ascontiguousarray(
            xt[:, :, 1:, :].transpose(2, 1, 0, 3)     # [sc-1, p, d, j]
        ).astype(ml_dtypes.bfloat16)
        in_maps.append({"wt": wt, "w0p": w0p, "xs0p": xs0p, "xs": xsb, "bi": bi})
    return in_maps


def _gather(results):
    outs = []
    for bb in range(B):
        o = np.asarray(results[bb]["out"]).astype(np.float32)
        outs.append(o.transpose(2, 0, 1).reshape(S, D))
    return np.ascontiguousarray(np.stack(outs))


def run(inputs, **spmd_kwargs):
    inputs = {k: np.asarray(v) for k, v in inputs.items()}
    in_maps = _prep_in_maps(**inputs)
    nc = _get_nc()
    res = run_bass_kernel_spmd(nc, in_maps, core_ids=list(range(B)), **spmd_kwargs)
    return _gather(res.results), res


def kernel(x, W, b, core0, core1, core2, core3, core4, core5):
    out, _ = run(dict(x=x, W=W, b=b, core0=core0, core1=core1, core2=core2,
                      core3=core3, core4=core4, core5=core5))
    return out


# revision 7
# speedup vs baseline: 1.0974x; 1.0256x over previous
"""TT-adapter linear kernel for TRN2, data-parallel over batch on 8 NeuronCores.

Math: out = x @ W.T + b + ALPHA * TT(x).  TT is linear in x, so the module
collapses to a single matmul with a merged weight folded on host:

    Wc = W + ALPHA * T          (T = TT-matrix reconstruction, 1024x1024)
    out = x @ Wc.T + b

The 34 GFLOP batched matmul runs on device in bf16 (f32 PSUM accumulation),
one batch element per NeuronCore, no collectives.  Raw bacc (manual
semaphores).  PE floor is 256 MMs x 216 ns = 55.3 us.

Measured DMA behavior that shapes the schedule: ONE HWDGE queue, in-order,
~400 GB/s streaming but ~0.45 us FIXED cost per DMA, and a DMA's completion
sem reaches +16 only 0.3-1.4 us after its data lands (16 per-engine incs
straggle).  Two queues split bandwidth without priority (bad).  So: fewest
possible DMAs, single SP queue, strictly in need-order, each granule sized
to what the PE staircase consumes per step.  The PE must also stay
continuously busy from the preamble until real data arrives (~3.4 us), else
the HAM clock-gate re-throttles and the first ~16 real MMs run at half rate.

Host layouts (per core, P=128 partitions, contraction dim on partitions):
    wxs0 bf16 [8, 128, 1544]  [d, p, 0:1024]    = Wc[:, 128d+p]  (all o)
                              [d, p, 1024:1536] = x[b, 0:512, 128d+p] (sc=0)
                              [0, p, 1536+oo]   = b[128oo+p]  (bias, bf16)
    xs   bf16 [3, 128, 8, 512] xs[i, p, d, j] = x[b, 512(i+1)+j, 128d+p]
    out  bf16 [8, 128, 2048]   out[oo, p, s]  = result[b, s, 128oo+p]

Schedule per core (group idx = 8*sc + o; bank = o; all MMs N=512):
  SP:  8 merged (w_d|x0_d|bias) DMAs in d order -- ONE gate per staircase
       step -- then sc=1 x in two halves (d0-3, d4-7), sc=2, sc=3; then
       out-DMAs idx=0..30 gated on evictions; final wait on slot sems.
  PE:  16 HAM-warm-up matmuls (~3.4 us, keeps the clock-gate warm until
       the first gate passes); phase 1 = sc=0 strip (o=0..7) d-outer
       staircase across all 8 PSUM banks; phase 2 = sc=1..3 strips,
       d-inner per group, each group's gate waits hoisted before the
       previous group's last MM so the NX resolves them while PE streams.
  ACT: dummy 8-col activate (hoists the lazy 1.3 us ACT_TABLE_LOAD into
       the preamble), 32 evictions (PSUM -> SBUF bf16 + bias add), last
       group's out-DMA ships from ACT directly (skips the SP sem hop).
"""

import numpy as np
import ml_dtypes
from contextlib import ExitStack

import concourse.bass as bass  # noqa: F401
import concourse.mybir as mybir
from concourse import bacc
from concourse.bass_utils import run_bass_kernel_spmd

ALPHA = 16.0
B, S, D = 8, 2048, 1024
P = 128
DO = D // P          # 8 contraction tiles
OO = D // P          # 8 output tiles
SCH = 512
NS = S // SCH        # 4 s-chunks
NG = OO * NS         # 32 groups
NBANK = 8
NSLOT = 8
WXC = D + SCH + 8    # merged per-d row: 1024 w | 512 x(sc0) | 8 bias
XOFF = D
BOFF = D + SCH

_NC = None


def _build_nc():
    nc = bacc.Bacc("TRN2", target_bir_lowering=False, debug=False)
    wxs0 = nc.declare_dram_parameter("wxs0", [DO, P, WXC], mybir.dt.bfloat16, isOutput=False)
    xs = nc.declare_dram_parameter("xs", [NS - 1, P, DO, SCH], mybir.dt.bfloat16, isOutput=False)
    out = nc.declare_dram_parameter("out", [OO, P, S], mybir.dt.bfloat16, isOutput=True)

    with ExitStack() as ctx:
        block = ctx.enter_context(nc.Block())
        # One sem per gating granule (HWDGE completions are unordered across
        # DMAs; each DMA incs its sem by 16, one per SDMA engine).
        s_wx = [ctx.enter_context(nc.semaphore(f"s_wx{d}")) for d in range(DO)]
        s_x1a = ctx.enter_context(nc.semaphore("s_x1a"))   # xs[0][:, 0:4]
        s_x1b = ctx.enter_context(nc.semaphore("s_x1b"))   # xs[0][:, 4:8]
        s_x2 = ctx.enter_context(nc.semaphore("s_x2"))
        s_x3 = ctx.enter_context(nc.semaphore("s_x3"))
        s_mm = ctx.enter_context(nc.semaphore("s_mm"))
        s_ev = ctx.enter_context(nc.semaphore("s_ev"))
        s_slot = [ctx.enter_context(nc.semaphore(f"s_slot{k}")) for k in range(NSLOT)]

        wx_sb = ctx.enter_context(nc.sbuf_tensor("wx_sb", [P, DO, WXC], mybir.dt.bfloat16))
        xb_sb = ctx.enter_context(nc.sbuf_tensor("xb_sb", [P, NS - 1, DO, SCH], mybir.dt.bfloat16))
        ot_sb = ctx.enter_context(nc.sbuf_tensor("ot_sb", [P, NSLOT, SCH], mybir.dt.bfloat16))
        ps = [ctx.enter_context(nc.psum_tensor(f"ps{b}", [P, SCH], mybir.dt.float32))
              for b in range(NBANK)]

        def wsl(o, d):
            return wx_sb[:, d, o * P:(o + 1) * P]

        def xsl(sc, d):
            if sc == 0:
                return wx_sb[:, d, XOFF:XOFF + SCH]
            return xb_sb[:, sc - 1, d, :]

        def bias_ap(o):
            return wx_sb[:, 0, BOFF + o:BOFF + o + 1]

        @block.sync
        def _(sync: bass.BassEngine):
            # strict need-order, one DMA per staircase step
            for d in range(DO):
                sync.dma_start(out=wx_sb[:, d, :], in_=wxs0[d]).then_inc(s_wx[d], 16)
            sync.dma_start(out=xb_sb[:, 0, 0:4, :], in_=xs[0][:, 0:4, :]).then_inc(s_x1a, 16)
            sync.dma_start(out=xb_sb[:, 0, 4:DO, :], in_=xs[0][:, 4:DO, :]).then_inc(s_x1b, 16)
            sync.dma_start(out=xb_sb[:, 1, :, :], in_=xs[1]).then_inc(s_x2, 16)
            sync.dma_start(out=xb_sb[:, 2, :, :], in_=xs[2]).then_inc(s_x3, 16)
            for g in range(NG - 1):
                o, sc = g % OO, g // OO
                sync.wait_ge(s_ev, g + 1)
                sync.dma_start(
                    out=out[o, :, sc * SCH:(sc + 1) * SCH],
                    in_=ot_sb[:, g % NSLOT, :],
                ).then_inc(s_slot[g % NSLOT], 16)
            for k in range(NSLOT):
                sync.wait_ge(s_slot[k], 16 * (NG // NSLOT))

        @block.tensor
        def _(tensor: bass.BassEngine):
            # HAM warm-up: ~3.4us of continuous dummy matmuls so the PE
            # clock-gate reaches 8/8 and STAYS there until the first real
            # gate passes; results discarded (bank 0 restarts, start=True).
            for _ in range(16):
                tensor.matmul(
                    ps[0][:, 0:256],
                    wx_sb[:, 0, 0:P],
                    wx_sb[:, 0, XOFF:XOFF + 256],
                    start=True,
                    stop=True,
                )
            # phase 1: sc=0 strip, d-outer staircase over banks 0..7 (=o)
            for d in range(DO):
                tensor.wait_ge(s_wx[d], 16)
                for o in range(OO):
                    mmi = tensor.matmul(
                        ps[o][:, :],
                        wsl(o, d),
                        xsl(0, d),
                        start=(d == 0),
                        stop=(d == DO - 1),
                    )
                    if d == DO - 1:
                        # d=7 octet runs in group order 0..7 -> s_mm incs
                        # arrive in the order the evictions expect
                        mmi.then_inc(s_mm, 1)
            # phase 2: sc=1..3 strips, d-inner per group.  Group g's gate
            # waits are emitted before the previous group's LAST matmul
            # (the waited-on eviction g-8 completed ~12us earlier, so this
            # only saves latency, never blocks the stream).
            for g in range(NBANK, NG):
                o, sc = g % OO, g // OO
                if g == NBANK:
                    tensor.wait_ge(s_x1a, 16)
                    tensor.wait_ge(s_ev, g - NBANK + 1)
                for d in range(DO):
                    if g == NBANK and d == 4:
                        tensor.wait_ge(s_x1b, 16)
                    if d == DO - 1 and g + 1 < NG:
                        no, nsc = (g + 1) % OO, (g + 1) // OO
                        if no == 0:
                            tensor.wait_ge(s_x2 if nsc == 2 else s_x3, 16)
                        tensor.wait_ge(s_ev, g + 1 - NBANK + 1)
                    mmi = tensor.matmul(
                        ps[o][:, :],
                        wsl(o, d),
                        xsl(sc, d),
                        start=(d == 0),
                        stop=(d == DO - 1),
                    )
                    if d == DO - 1:
                        mmi.then_inc(s_mm, 1)

        @block.scalar
        def _(scalar: bass.BassEngine):
            # dummy 8-col activate: pulls the lazy ACT_TABLE_LOAD into the
            # preamble window (it otherwise delays the first real eviction
            # by ~1.3us).  Reads garbage; slot 0 is fully overwritten by
            # eviction 0 before any out-DMA reads it.
            scalar.add(ot_sb[:, 0, 0:8], ot_sb[:, 1, 0:8], 0.0)
            for g in range(NG):
                o, sc = g % OO, g // OO
                if g == 0:
                    scalar.wait_ge(s_wx[0], 16)   # bias rides in wxs0[0]
                scalar.wait_ge(s_mm, g + 1)
                if g >= NSLOT:
                    scalar.wait_ge(s_slot[g % NSLOT], 16 * (g // NSLOT))
                scalar.add(
                    ot_sb[:, g % NSLOT, :], ps[o][:, :], bias_ap(o)
                ).then_inc(s_ev, 1)
                if g == NG - 1:
                    # last output ships from ACT (also HWDGE, its own queue):
                    # skips the SP semaphore hop on the critical tail
                    scalar.dma_start(
                        out=out[o, :, sc * SCH:(sc + 1) * SCH],
                        in_=ot_sb[:, g % NSLOT, :],
                    ).then_inc(s_slot[g % NSLOT], 16)

    nc.compile()
    return nc


def _get_nc():
    global _NC
    if _NC is None:
        _NC = _build_nc()
    return _NC


def _merged_weight_T(W, b, core0, core1, core2, core3, core4, core5):
    f8 = np.float64
    A = core0[0].astype(f8)
    Bm = np.einsum('ap,pbq->abq', A, core1.astype(f8))
    C = np.einsum('abq,qcr->abcr', Bm, core2.astype(f8))
    Phi = C.transpose(2, 1, 0, 3).reshape(D, 8)
    Dn = np.einsum('paq,qbr->pabr', core3.astype(f8), core4.astype(f8))
    E = np.einsum('pabq,qc->pabc', Dn, core5[:, :, 0].astype(f8))
    Psi = E.reshape(8, D)
    WcT = W.T.astype(f8) + ALPHA * (Phi @ Psi)
    return WcT.astype(np.float32)


def _prep_in_maps(x, W, b, core0, core1, core2, core3, core4, core5):
    WcT = _merged_weight_T(W, b, core0, core1, core2, core3, core4, core5)
    wt16 = WcT.reshape(DO, P, D).astype(ml_dtypes.bfloat16)
    bias_pad = np.zeros((DO, P, 8), dtype=ml_dtypes.bfloat16)
    bias_pad[0] = b.reshape(OO, P).T.astype(ml_dtypes.bfloat16)
    in_maps = []
    for bb in range(B):
        xt = x[bb].T.reshape(DO, P, NS, SCH)          # [d, p, sc, j]
        x0 = xt[:, :, 0, :].astype(ml_dtypes.bfloat16)
        wxs0 = np.ascontiguousarray(
            np.concatenate([wt16, x0, bias_pad], axis=2))
        xsb = np.ascontiguousarray(
            xt[:, :, 1:, :].transpose(2, 1, 0, 3)     # [sc-1, p, d, j]
        ).astype(ml_dtypes.bfloat16)
        in_maps.append({"wxs0": wxs0, "xs": xsb})
    return in_maps


def _gather(results):
    outs = []
    for bb in range(B):
        o = np.asarray(results[bb]["out"]).astype(np.float32)
        outs.append(o.transpose(2, 0, 1).reshape(S, D))
    return np.ascontiguousarray(np.stack(outs))


def run(inputs, **spmd_kwargs):
    inputs = {k: np.asarray(v) for k, v in inputs.items()}
    in_maps = _prep_in_maps(**inputs)
    nc = _get_nc()
    res = run_bass_kernel_spmd(nc, in_maps, core_ids=list(range(B)), **spmd_kwargs)
    return _gather(res.results), res


def kernel(x, W, b, core0, core1, core2, core3, core4, core5):
    out, _ = run(dict(x=x, W=W, b=b, core0=core0, core1=core1, core2=core2,
                      core3=core3, core4=core4, core5=core5))
    return out


# revision 10
# speedup vs baseline: 1.1009x; 1.0032x over previous
"""TT-adapter linear kernel for TRN2, data-parallel over batch on 8 NeuronCores.

Math: out = x @ W.T + b + ALPHA * TT(x).  TT is linear in x, so the module
collapses to a single matmul with a merged weight folded on host:

    Wc = W + ALPHA * T          (T = TT-matrix reconstruction, 1024x1024)
    out = x @ Wc.T + b

The 34 GFLOP batched matmul runs on device in bf16 (f32 PSUM accumulation),
one batch element per NeuronCore, no collectives.  Raw bacc (manual
semaphores).  PE floor is 256 MMs x 216 ns = 55.3 us.

Measured DMA behavior that shapes the schedule: ONE HWDGE queue, in-order,
~400 GB/s streaming but ~0.45 us FIXED cost per DMA, and a DMA's completion
sem reaches +16 only 0.3-1.4 us after its data lands (16 per-engine incs
straggle).  Two queues split bandwidth without priority (bad).  So: fewest
possible DMAs, single SP queue, strictly in need-order, each granule sized
to what the PE staircase consumes per step.  The PE must also stay
continuously busy from the preamble until real data arrives (~3.4 us), else
the HAM clock-gate re-throttles and the first ~16 real MMs run at half rate.

Host layouts (per core, P=128 partitions, contraction dim on partitions):
    wxs0 bf16 [8, 128, 1544]  [d, p, 0:1024]    = Wc[:, 128d+p]  (all o)
                              [d, p, 1024:1536] = x[b, 0:512, 128d+p] (sc=0)
                              [0, p, 1536+oo]   = b[128oo+p]  (bias, bf16)
    xs   bf16 [3, 128, 8, 512] xs[i, p, d, j] = x[b, 512(i+1)+j, 128d+p]
    out  bf16 [8, 128, 2048]   out[oo, p, s]  = result[b, s, 128oo+p]

Schedule per core (group idx = 8*sc + o; bank = o; all MMs N=512):
  SP:  8 merged (w_d|x0_d|bias) DMAs in d order -- ONE gate per staircase
       step -- then sc=1 x in two halves (d0-3, d4-7), sc=2, sc=3; then
       out-DMAs idx=0..30 gated on evictions; final wait on slot sems.
  PE:  16 HAM-warm-up matmuls (~3.4 us, keeps the clock-gate warm until
       the first gate passes); phase 1 = sc=0 strip (o=0..7) d-outer
       staircase across all 8 PSUM banks; phase 2 = sc=1..3 strips,
       d-inner per group, each group's gate waits hoisted before the
       previous group's last MM so the NX resolves them while PE streams.
  ACT: dummy 8-col activate (hoists the lazy 1.3 us ACT_TABLE_LOAD into
       the preamble), 32 evictions (PSUM -> SBUF bf16 + bias add), last
       group's out-DMA ships from ACT directly (skips the SP sem hop).
"""

import numpy as np
import ml_dtypes
from contextlib import ExitStack

import concourse.bass as bass  # noqa: F401
import concourse.mybir as mybir
from concourse import bacc
from concourse.bass_utils import run_bass_kernel_spmd

ALPHA = 16.0
B, S, D = 8, 2048, 1024
P = 128
DO = D // P          # 8 contraction tiles
OO = D // P          # 8 output tiles
SCH = 512
NS = S // SCH        # 4 s-chunks
NG = OO * NS         # 32 groups
NBANK = 8
NSLOT = 4
WXC = D + SCH + 8    # merged per-d row: 1024 w | 512 x(sc0) | 8 bias
XOFF = D
BOFF = D + SCH

_NC = None


def _build_nc():
    nc = bacc.Bacc("TRN2", target_bir_lowering=False, debug=False)
    wxs0 = nc.declare_dram_parameter("wxs0", [DO, P, WXC], mybir.dt.bfloat16, isOutput=False)
    xs = nc.declare_dram_parameter("xs", [P, NS - 1, DO, SCH], mybir.dt.bfloat16, isOutput=False)
    out = nc.declare_dram_parameter("out", [OO, P, S], mybir.dt.bfloat16, isOutput=True)

    with ExitStack() as ctx:
        block = ctx.enter_context(nc.Block())
        # One sem per gating granule (HWDGE completions are unordered across
        # DMAs; each DMA incs its sem by 16, one per SDMA engine).
        s_wx = [ctx.enter_context(nc.semaphore(f"s_wx{d}")) for d in range(DO)]
        s_x1 = ctx.enter_context(nc.semaphore("s_x1"))     # xs[0]  (sc=1)
        s_x23 = ctx.enter_context(nc.semaphore("s_x23"))   # xs[1:] (sc=2,3)
        s_mm = ctx.enter_context(nc.semaphore("s_mm"))
        s_ev = ctx.enter_context(nc.semaphore("s_ev"))
        s_slot = [ctx.enter_context(nc.semaphore(f"s_slot{k}")) for k in range(NSLOT)]

        wx_sb = ctx.enter_context(nc.sbuf_tensor("wx_sb", [P, DO, WXC], mybir.dt.bfloat16))
        xb_sb = ctx.enter_context(nc.sbuf_tensor("xb_sb", [P, NS - 1, DO, SCH], mybir.dt.bfloat16))
        ot_sb = ctx.enter_context(nc.sbuf_tensor("ot_sb", [P, NSLOT, SCH], mybir.dt.bfloat16))
        ps = [ctx.enter_context(nc.psum_tensor(f"ps{b}", [P, SCH], mybir.dt.float32))
              for b in range(NBANK)]

        def wsl(o, d):
            return wx_sb[:, d, o * P:(o + 1) * P]

        def xsl(sc, d):
            if sc == 0:
                return wx_sb[:, d, XOFF:XOFF + SCH]
            return xb_sb[:, sc - 1, d, :]

        def bias_ap(o):
            return wx_sb[:, 0, BOFF + o:BOFF + o + 1]

        @block.sync
        def _(sync: bass.BassEngine):
            # strict need-order, one DMA per staircase step
            for d in range(DO):
                sync.dma_start(out=wx_sb[:, d, :], in_=wxs0[d]).then_inc(s_wx[d], 16)
            sync.dma_start(out=xb_sb[:, 0, :, :], in_=xs[:, 0, :, :]).then_inc(s_x1, 16)
            sync.dma_start(out=xb_sb[:, 1:, :, :], in_=xs[:, 1:, :, :]).then_inc(s_x23, 16)
            for g in range(NG - 1):
                o, sc = g % OO, g // OO
                sync.wait_ge(s_ev, g + 1)
                sync.dma_start(
                    out=out[o, :, sc * SCH:(sc + 1) * SCH],
                    in_=ot_sb[:, g % NSLOT, :],
                ).then_inc(s_slot[g % NSLOT], 16)
            for k in range(NSLOT):
                sync.wait_ge(s_slot[k], 16 * (NG // NSLOT))

        @block.tensor
        def _(tensor: bass.BassEngine):
            # HAM warm-up: ~3.4us of continuous dummy matmuls so the PE
            # clock-gate reaches 8/8 and STAYS there until the first real
            # gate passes; results discarded (bank 0 restarts, start=True).
            for _ in range(16):
                tensor.matmul(
                    ps[0][:, 0:256],
                    wx_sb[:, 0, 0:P],
                    wx_sb[:, 0, XOFF:XOFF + 256],
                    start=True,
                    stop=True,
                )
            # phase 1: sc=0 strip, d-outer staircase over banks 0..7 (=o).
            # The NEXT step's gate wait is hoisted before each step's last
            # MM so the NX resolves it while the PE streams (d+1's DMA sem
            # never depends on the PE, so this cannot deadlock).
            for d in range(DO):
                if d == 0:
                    tensor.wait_ge(s_wx[0], 16)
                for o in range(OO):
                    if o == OO - 1:
                        if d < DO - 1:
                            tensor.wait_ge(s_wx[d + 1], 16)
                        else:
                            # phase-2 g=8 gates: ev(0) completes ~0.2us
                            # before this MM would issue (s_mm(1) fired 7
                            # MMs ago), so this wait is already satisfied
                            tensor.wait_ge(s_x1, 16)
                            tensor.wait_ge(s_ev, 1)
                    mmi = tensor.matmul(
                        ps[o][:, :],
                        wsl(o, d),
                        xsl(0, d),
                        start=(d == 0),
                        stop=(d == DO - 1),
                    )
                    if d == DO - 1:
                        # d=7 octet runs in group order 0..7 -> s_mm incs
                        # arrive in the order the evictions expect
                        mmi.then_inc(s_mm, 1)
            # phase 2: sc=1..3 strips, d-inner per group.  Group g's gate
            # waits are emitted before the previous group's LAST matmul
            # (the waited-on eviction g-8 completed ~12us earlier, so this
            # only saves latency, never blocks the stream).
            for g in range(NBANK, NG):
                o, sc = g % OO, g // OO
                for d in range(DO):
                    if d == DO - 1 and g + 1 < NG:
                        no, nsc = (g + 1) % OO, (g + 1) // OO
                        if no == 0 and nsc == 2:
                            tensor.wait_ge(s_x23, 16)
                        tensor.wait_ge(s_ev, g + 1 - NBANK + 1)
                    mmi = tensor.matmul(
                        ps[o][:, :],
                        wsl(o, d),
                        xsl(sc, d),
                        start=(d == 0),
                        stop=(d == DO - 1),
                    )
                    if d == DO - 1:
                        mmi.then_inc(s_mm, 1)

        @block.scalar
        def _(scalar: bass.BassEngine):
            # dummy 8-col activate: pulls the lazy ACT_TABLE_LOAD into the
            # preamble window (it otherwise delays the first real eviction
            # by ~1.3us).  Reads garbage; slot 0 is fully overwritten by
            # eviction 0 before any out-DMA reads it.
            scalar.add(ot_sb[:, 0, 0:8], ot_sb[:, 1, 0:8], 0.0)
            for g in range(NG):
                o, sc = g % OO, g // OO
                if g == 0:
                    scalar.wait_ge(s_wx[0], 16)   # bias rides in wxs0[0]
                scalar.wait_ge(s_mm, g + 1)
                if g >= NSLOT:
                    scalar.wait_ge(s_slot[g % NSLOT], 16 * (g // NSLOT))
                scalar.add(
                    ot_sb[:, g % NSLOT, :], ps[o][:, :], bias_ap(o)
                ).then_inc(s_ev, 1)
                if g == NG - 1:
                    # last output ships from ACT (also HWDGE, its own queue):
                    # skips the SP semaphore hop on the critical tail
                    scalar.dma_start(
                        out=out[o, :, sc * SCH:(sc + 1) * SCH],
                        in_=ot_sb[:, g % NSLOT, :],
                    ).then_inc(s_slot[g % NSLOT], 16)

    nc.compile()
    return nc


def _get_nc():
    global _NC
    if _NC is None:
        _NC = _build_nc()
    return _NC


def _merged_weight_T(W, b, core0, core1, core2, core3, core4, core5):
    f8 = np.float64
    A = core0[0].astype(f8)
    Bm = np.einsum('ap,pbq->abq', A, core1.astype(f8))
    C = np.einsum('abq,qcr->abcr', Bm, core2.astype(f8))
    Phi = C.transpose(2, 1, 0, 3).reshape(D, 8)
    Dn = np.einsum('paq,qbr->pabr', core3.astype(f8), core4.astype(f8))
    E = np.einsum('pabq,qc->pabc', Dn, core5[:, :, 0].astype(f8))
    Psi = E.reshape(8, D)
    WcT = W.T.astype(f8) + ALPHA * (Phi @ Psi)
    return WcT.astype(np.float32)


def _prep_in_maps(x, W, b, core0, core1, core2, core3, core4, core5):
    WcT = _merged_weight_T(W, b, core0, core1, core2, core3, core4, core5)
    wt16 = WcT.reshape(DO, P, D).astype(ml_dtypes.bfloat16)
    bias_pad = np.zeros((DO, P, 8), dtype=ml_dtypes.bfloat16)
    bias_pad[0] = b.reshape(OO, P).T.astype(ml_dtypes.bfloat16)
    in_maps = []
    for bb in range(B):
        xt = x[bb].T.reshape(DO, P, NS, SCH)          # [d, p, sc, j]
        x0 = xt[:, :, 0, :].astype(ml_dtypes.bfloat16)
        wxs0 = np.ascontiguousarray(
            np.concatenate([wt16, x0, bias_pad], axis=2))
        xsb = np.ascontiguousarray(
            xt[:, :, 1:, :].transpose(1, 2, 0, 3)     # [p, sc-1, d, j]
        ).astype(ml_dtypes.bfloat16)
        in_maps.append({"wxs0": wxs0, "xs": xsb})
    return in_maps


def _gather(results):
    outs = []
    for bb in range(B):
        o = np.asarray(results[bb]["out"]).astype(np.float32)
        outs.append(o.transpose(2, 0, 1).reshape(S, D))
    return np.ascontiguousarray(np.stack(outs))


def run(inputs, **spmd_kwargs):
    inputs = {k: np.asarray(v) for k, v in inputs.items()}
    in_maps = _prep_in_maps(**inputs)
    nc = _get_nc()
    res = run_bass_kernel_spmd(nc, in_maps, core_ids=list(range(B)), **spmd_kwargs)
    return _gather(res.results), res


def kernel(x, W, b, core0, core1, core2, core3, core4, core5):
    out, _ = run(dict(x=x, W=W, b=b, core0=core0, core1=core1, core2=core2,
                      core3=core3, core4=core4, core5=core5))
    return out
